# revision 3
# baseline (speedup 1.0000x reference)
"""Mistral sparse-MoE (B=4,S=2048,H=1024,F=4096,E=8,top-2) on 8 trn2 cores.

Expert-parallel sharding: core e holds expert e's gate/up/down weights.
The host computes the (tiny) router + top-2 dispatch and uses it to shard:
each core receives exactly the tokens routed to its expert (gathered,
transposed, zero-padded to a common capacity C), the expert weights in
K-major bf16 layout, and the per-token combine weights. The device kernel
computes the full expert FFN  y = (silu(x@gW^T) * (x@uW^T)) @ dW^T * w
for its tokens; the host scatter-adds the 8 partial outputs back into the
[T, H] result (pure unshard of the expert-parallel partial sums).
"""

import numpy as np
import ml_dtypes
from contextlib import ExitStack

B, S, H, F, E, TOPK = 4, 2048, 1024, 4096, 8, 2
T = B * S
P = 128
NCH = 512          # token chunk (columns per psum tile)
FB = 512           # F-stripe width loaded per weight DMA
KH = H // P        # 8  contraction chunks for gate/up
KF = F // P        # 32 contraction chunks for down
HM = H // P        # 8  output row tiles

_BF16 = ml_dtypes.bfloat16


def _build_program(C):
    import concourse.tile as tile
    from concourse import bacc, mybir

    bf16 = mybir.dt.bfloat16
    f32 = mybir.dt.float32

    nc = bacc.Bacc("TRN2", target_bir_lowering=False, debug=False, num_devices=E)

    xT = nc.dram_tensor("xT", [H, C], bf16, kind="ExternalInput").ap()
    gw = nc.dram_tensor("gw", [H, F], bf16, kind="ExternalInput").ap()
    uw = nc.dram_tensor("uw", [H, F], bf16, kind="ExternalInput").ap()
    dw = nc.dram_tensor("dw", [F, H], bf16, kind="ExternalInput").ap()
    wr = nc.dram_tensor("wr", [P, C], f32, kind="ExternalInput").ap()
    yT = nc.dram_tensor("yT", [H, C], f32, kind="ExternalOutput").ap()

    chunks = []
    n0 = 0
    while n0 < C:
        nn = min(NCH, C - n0)
        chunks.append((n0, nn))
        n0 += nn

    with tile.TileContext(nc) as tc, ExitStack() as ctx:
        dwp = ctx.enter_context(tc.tile_pool(name="dwp", bufs=1))
        wp = ctx.enter_context(tc.tile_pool(name="wp", bufs=1))
        xp = ctx.enter_context(tc.tile_pool(name="xp", bufs=2))
        gwp = ctx.enter_context(tc.tile_pool(name="gwp", bufs=2))
        uwp = ctx.enter_context(tc.tile_pool(name="uwp", bufs=2))
        hp = ctx.enter_context(tc.tile_pool(name="hp", bufs=2))
        sgp = ctx.enter_context(tc.tile_pool(name="sgp", bufs=3))
        yp = ctx.enter_context(tc.tile_pool(name="yp", bufs=3))
        pg = ctx.enter_context(tc.tile_pool(name="pg", bufs=2, space="PSUM"))
        pu = ctx.enter_context(tc.tile_pool(name="pu", bufs=2, space="PSUM"))
        py = ctx.enter_context(tc.tile_pool(name="py", bufs=2, space="PSUM"))

        # down-proj weights stay resident: 32 tiles [128, 1024] bf16 = 8MB
        dwt = []
        for k in range(KF):
            t = dwp.tile([P, H], bf16, tag=f"dw{k}")
            nc.sync.dma_start(out=t[:], in_=dw[k * P:(k + 1) * P, :])
            dwt.append(t)
        wt = wp.tile([P, C], f32)
        nc.sync.dma_start(out=wt[:], in_=wr[:, :])

        for (n0, nn) in chunks:
            xts = []
            for k in range(KH):
                t = xp.tile([P, nn], bf16, tag=f"x{k}")
                nc.sync.dma_start(out=t[:], in_=xT[k * P:(k + 1) * P, n0:n0 + nn])
                xts.append(t)

            hts = []
            for fb in range(F // FB):
                gts, uts = [], []
                for k in range(KH):
                    gt = gwp.tile([P, FB], bf16, tag=f"g{k}")
                    nc.sync.dma_start(out=gt[:], in_=gw[k * P:(k + 1) * P, fb * FB:(fb + 1) * FB])
                    ut = uwp.tile([P, FB], bf16, tag=f"u{k}")
                    nc.sync.dma_start(out=ut[:], in_=uw[k * P:(k + 1) * P, fb * FB:(fb + 1) * FB])
                    gts.append(gt)
                    uts.append(ut)
                for fm in range(FB // P):
                    j = fb * (FB // P) + fm
                    psg = pg.tile([P, nn], f32)
                    psu = pu.tile([P, nn], f32)
                    for k in range(KH):
                        nc.tensor.matmul(
                            psg[:], gts[k][:, fm * P:(fm + 1) * P], xts[k][:],
                            start=(k == 0), stop=(k == KH - 1))
                    for k in range(KH):
                        nc.tensor.matmul(
                            psu[:], uts[k][:, fm * P:(fm + 1) * P], xts[k][:],
                            start=(k == 0), stop=(k == KH - 1))
                    sg = sgp.tile([P, nn], bf16)
                    nc.scalar.activation(
                        sg[:], psg[:], mybir.ActivationFunctionType.Silu)
                    ht = hp.tile([P, nn], bf16, tag=f"h{j}")
                    nc.vector.tensor_mul(ht[:], sg[:], psu[:])
                    hts.append(ht)

            for hm in range(HM):
                psy = py.tile([P, nn], f32)
                for k in range(KF):
                    nc.tensor.matmul(
                        psy[:], dwt[k][:, hm * P:(hm + 1) * P], hts[k][:],
                        start=(k == 0), stop=(k == KF - 1))
                ys = yp.tile([P, nn], f32)
                nc.vector.tensor_mul(ys[:], psy[:], wt[:, n0:n0 + nn])
                nc.sync.dma_start(out=yT[hm * P:(hm + 1) * P, n0:n0 + nn], in_=ys[:])

    nc.finalize()
    return nc


def _route(x, router_w):
    # top-2 routing in f64 (exactly ties-stable vs the fp32 reference for
    # any non-degenerate logits)
    logits = x.astype(np.float64) @ router_w.T.astype(np.float64)
    rows = np.arange(T)
    i1 = np.argmax(logits, axis=1)
    v1 = logits[rows, i1]
    masked = logits.copy()
    masked[rows, i1] = -np.inf
    i2 = np.argmax(masked, axis=1)
    v2 = masked[rows, i2]
    e2 = np.exp(v2 - v1)
    w1 = 1.0 / (1.0 + e2)
    w2 = e2 / (1.0 + e2)
    return i1, i2, w1.astype(np.float32), w2.astype(np.float32)


def kernel(hidden_states, router_w, gate_w, up_w, down_w):
    from concourse.bass_utils import run_bass_kernel_spmd

    x = np.asarray(hidden_states, dtype=np.float32).reshape(T, H)
    router_w = np.asarray(router_w, dtype=np.float32)

    i1, i2, w1, w2 = _route(x, router_w)

    idxs, wts = [], []
    for e in range(E):
        m1 = i1 == e
        m2 = i2 == e
        idx = np.nonzero(m1 | m2)[0]
        w = np.where(m1[idx], w1[idx], w2[idx])
        idxs.append(idx)
        wts.append(w)

    max_ne = max(len(i) for i in idxs)
    C = max(NCH, ((max_ne + P - 1) // P) * P)

    x_bf = x.astype(_BF16)
    in_maps = []
    for e in range(E):
        idx, w = idxs[e], wts[e]
        n_e = len(idx)
        xTe = np.zeros((H, C), dtype=_BF16)
        xTe[:, :n_e] = x_bf[idx].T
        wre = np.zeros((P, C), dtype=np.float32)
        wre[:, :n_e] = w[None, :]
        in_maps.append({
            "xT": xTe,
            "gw": np.ascontiguousarray(np.asarray(gate_w)[e].T).astype(_BF16),
            "uw": np.ascontiguousarray(np.asarray(up_w)[e].T).astype(_BF16),
            "dw": np.ascontiguousarray(np.asarray(down_w)[e].T).astype(_BF16),
            "wr": wre,
        })

    nc = _build_program(C)
    results = run_bass_kernel_spmd(nc, in_maps, list(range(E))).results

    out = np.zeros((T, H), dtype=np.float32)
    for e in range(E):
        idx = idxs[e]
        out[idx] += results[e]["yT"][:, :len(idx)].T
    return out.reshape(B, S, H)
